# revision 2
# baseline (speedup 1.0000x reference)
"""Trainium2 kernel for nn_CrossLayerLateral.

out[b,s,i] = x_current[b,s,i] + alpha * sum_j x_prev[b,s,j] * W[i,j]
with W built from COO (duplicates summed).

Strategy (data-parallel over tokens, 8 cores):
  - Host: build WT[j,i] = alpha * W[i,j] via bincount scatter-add (cheap,
    O(nnz)); cast WT and x_prev to bf16 (error lands on the alpha-scaled
    lateral term only: ~0.4% of 1.4e-4 of |out| => ~1e-6 relative).
  - Flatten (B,S) -> 8192 tokens, 1024 tokens per core. Each core gets:
      xc  [1024, 2048] f32   x_current slice (token-major)
      xpt [16, 128, 1024] bf16  x_prev slice transposed (j on partitions)
      wt  [16, 128, 2048] bf16  alpha * W^T, replicated
  - Device: out = xc + xpt.T @ wt accumulated over 16 j-chunks in PSUM
    (fp32), DVE adds xc, DMA out.
"""
import numpy as np
import ml_dtypes

import concourse.bass as bass
import concourse.tile as tile
from concourse import bacc, mybir
from concourse.bass_utils import run_bass_kernel_spmd

H = 2048          # hidden
B, S = 4, 2048
TOK = B * S       # 8192 tokens
NCORES = 8
TPC = TOK // NCORES   # 1024 tokens per core
P = 128
JC = H // P       # 16 j-chunks (contraction)
ST = TPC // P     # 8 token tiles per core
NB = 512          # matmul free dim / psum bank
IB = H // NB      # 4 output column blocks

_NC_CACHE = {}


def build_nc():
    nc = bacc.Bacc("TRN2", target_bir_lowering=False, debug=False,
                   num_devices=NCORES)
    xc = nc.dram_tensor("xc", [TPC, H], mybir.dt.float32,
                        kind="ExternalInput").ap()
    xpt = nc.dram_tensor("xpt", [JC, P, TPC], mybir.dt.bfloat16,
                         kind="ExternalInput").ap()
    wt = nc.dram_tensor("wt", [JC, P, H], mybir.dt.bfloat16,
                        kind="ExternalInput").ap()
    out = nc.dram_tensor("out", [TPC, H], mybir.dt.float32,
                         kind="ExternalOutput").ap()

    with tile.TileContext(nc) as tc:
        with (
            tc.tile_pool(name="weights", bufs=1) as wpool,
            tc.tile_pool(name="acts", bufs=1) as xpool,
            tc.tile_pool(name="io", bufs=3) as io,
            tc.tile_pool(name="psum", bufs=2, space="PSUM") as psum,
        ):
            # Resident weights + transposed activations, chunked per
            # j-block so matmuls can start as soon as their chunk lands.
            wt_sb = []
            xpt_sb = []
            for jo in range(JC):
                w = wpool.tile([P, H], mybir.dt.bfloat16, name=f"w{jo}",
                               tag=f"w{jo}")
                nc.sync.dma_start(w[:], wt[jo])
                wt_sb.append(w)
                x = xpool.tile([P, TPC], mybir.dt.bfloat16, name=f"x{jo}",
                               tag=f"x{jo}")
                nc.sync.dma_start(x[:], xpt[jo])
                xpt_sb.append(x)

            for st in range(ST):
                ssl = bass.ts(st, P)
                xc_t = io.tile([P, H], mybir.dt.float32, name=f"xc{st}",
                               tag="xc")
                nc.sync.dma_start(xc_t[:], xc[ssl, :])
                out_t = io.tile([P, H], mybir.dt.float32, name=f"o{st}",
                                tag="out")
                ps = [psum.tile([P, NB], mybir.dt.float32, name=f"ps{st}_{ib}",
                                tag=f"ps{ib}") for ib in range(IB)]
                # Same stationary operand (xpt chunk) for 4 consecutive
                # matmuls into 4 psum banks; accumulate over j-chunks.
                for jo in range(JC):
                    lhsT = xpt_sb[jo][:, ssl]
                    for ib in range(IB):
                        nc.tensor.matmul(
                            ps[ib][:], lhsT=lhsT,
                            rhs=wt_sb[jo][:, bass.ts(ib, NB)],
                            start=(jo == 0), stop=(jo == JC - 1),
                        )
                for ib in range(IB):
                    isl = bass.ts(ib, NB)
                    nc.vector.tensor_add(out_t[:, isl], ps[ib][:],
                                         xc_t[:, isl])
                nc.sync.dma_start(out[ssl, :], out_t[:])

    nc.compile()
    return nc


def _get_nc():
    if "nc" not in _NC_CACHE:
        _NC_CACHE["nc"] = build_nc()
    return _NC_CACHE["nc"]


def _prep_inputs(x_current, x_prev, alpha, connection_values,
                 connection_indices):
    # WT[j, i] = alpha * W[i, j];  W[r, c] += v  =>  WT[c, r] += v
    r = connection_indices[0].astype(np.int64)
    c = connection_indices[1].astype(np.int64)
    wt_flat = np.bincount(c * H + r, weights=connection_values.astype(np.float64),
                          minlength=H * H)
    wt = (np.float32(alpha) * wt_flat.astype(np.float32))
    wt_bf = wt.astype(ml_dtypes.bfloat16).reshape(JC, P, H)

    xp = np.asarray(x_prev, dtype=np.float32).reshape(TOK, H)
    xc = np.ascontiguousarray(np.asarray(x_current,
                                         dtype=np.float32).reshape(TOK, H))
    in_maps = []
    for core in range(NCORES):
        sl = slice(core * TPC, (core + 1) * TPC)
        xpt_core = xp[sl].T.astype(ml_dtypes.bfloat16).reshape(JC, P, TPC)
        in_maps.append({
            "xc": np.ascontiguousarray(xc[sl]),
            "xpt": np.ascontiguousarray(xpt_core),
            "wt": wt_bf,
        })
    return in_maps


def kernel(x_current, x_prev, alpha, connection_values, connection_indices):
    nc = _get_nc()
    in_maps = _prep_inputs(x_current, x_prev, alpha, connection_values,
                           connection_indices)
    res = run_bass_kernel_spmd(nc, in_maps, list(range(NCORES)))
    out = np.concatenate([res.results[i]["out"] for i in range(NCORES)], axis=0)
    return out.reshape(B, S, H).astype(np.float32, copy=False)
